# revision 1
# baseline (speedup 1.0000x reference)
"""Causal attention with RoPE on 8 Trainium2 NeuronCores.

Tensor-parallel over heads: core c owns heads [2c, 2c+2). Each core computes
its heads' Q/K/V projections (fp32r matmuls), RoPE, causal attention in a
transposed layout (keys on partitions), and a partial output projection
through its slice of Wo. The 8 partial outputs are summed on the host.

Layout notes:
  - x is passed transposed (xT [D, B*S]) so d_model lands on partitions for
    all projection matmuls.
  - Q/K are produced transposed (QT/KT [head_dim, S]); scores are computed
    transposed (scoresT [k, q]) so the attn@V contraction has keys on
    partitions for both operands. Softmax denominators come from an M=1
    matmul with a ones vector; normalization happens on the attention output
    tiles via a gpsimd partition-broadcast of 1/denom.
  - All matmul inputs are float32r (fp32 truncated to fp22 by the PE), which
    streams at full speed (1 cycle/row) instead of fp32's 1/4 rate.
  - The causal structure lets attention for query block qj start as soon as
    projections for column block cb=qj are done, so each iteration runs
    proj(cb) -> attention(qj=cb) -> output rows of qj; engines stay mixed
    and the DMA-paced warmup overlaps compute.
"""
import numpy as np
import ml_dtypes

import concourse.bacc as bacc
import concourse.bass as bass
import concourse.tile as tile
import concourse.mybir as mybir
from concourse.bass_utils import run_bass_kernel_spmd

AF = mybir.ActivationFunctionType
F32 = mybir.dt.float32
F32R = mybir.dt.float32r
BF16 = mybir.dt.bfloat16

P = 128            # partitions
DH = 128           # head dim
D = 2048           # d_model
S = 2048           # sequence length
B = 2              # batch
NCORES = 8
HL = 2             # heads per core
LF = HL * DH       # 256 local head features
KC = D // P        # 16 d_model chunks
NCB = S // 512     # 4 column blocks of 512 positions per batch
NKB = S // P       # 16 key blocks per batch
NNT = D // 512     # 4 output column tiles
ROWS = B * S
SCALE = float(1.0 / np.sqrt(DH))

_PROG = None


def _emit_consts(nc, sbp, t):
    wqT, wkT, wvT, woT, cosT, sinT, bandT, onesT = (
        t["wqT"], t["wkT"], t["wvT"], t["woT"], t["cosT"], t["sinT"],
        t["bandT"], t["onesT"])
    wq = sbp.tile([P, KC * LF], BF16, name="wq")
    wk = sbp.tile([P, KC * LF], BF16, name="wk")
    wv = sbp.tile([P, KC * LF], BF16, name="wv")
    wo = sbp.tile([P, HL * D], BF16, name="wo")
    cos = sbp.tile([DH, S], F32, name="cos")
    sin = sbp.tile([DH, S], F32, name="sin")
    band = sbp.tile([P, P], BF16, name="band")
    ones = sbp.tile([P, 1], BF16, name="ones")
    for g in range(4):
        gk = slice(g * 4 * P, (g + 1) * 4 * P)
        nc.sync.dma_start(
            out=wq[:, g * 4 * LF:(g + 1) * 4 * LF],
            in_=wqT[gk, :].rearrange("(kc p) f -> p kc f", p=P))
        nc.scalar.dma_start(
            out=wk[:, g * 4 * LF:(g + 1) * 4 * LF],
            in_=wkT[gk, :].rearrange("(kc p) f -> p kc f", p=P))
        nc.gpsimd.dma_start(
            out=wv[:, g * 4 * LF:(g + 1) * 4 * LF],
            in_=wvT[gk, :].rearrange("(kc p) f -> p kc f", p=P))
    nc.scalar.dma_start(out=cos, in_=cosT[:, :])
    nc.scalar.dma_start(out=sin, in_=sinT[:, :])
    nc.gpsimd.dma_start(out=band, in_=bandT[:, :])
    nc.gpsimd.dma_start(out=ones, in_=onesT[:, :])
    for h in range(HL):
        nc.gpsimd.dma_start(
            out=wo[:, h * D:(h + 1) * D],
            in_=woT[h * P:(h + 1) * P, :])
    return dict(wq=wq, wk=wk, wv=wv, wo=wo, cos=cos, sin=sin, band=band,
                ones=ones)


def _emit(nc, sbp, psp, c, t):
    xT, out = t["xT"], t["out"]
    wq, wk, wv, cos, sin = c["wq"], c["wk"], c["wv"], c["cos"], c["sin"]

    emit_wo = _make_emit_wo(nc, sbp, psp, c["wo"], out)
    emit_attn = _make_emit_attn(nc, sbp, psp, c["band"], c["ones"])

    for b in range(B):
        qt = sbp.tile([P, HL * S], BF16, name=f"qt{b}", tag="qt")
        kt = sbp.tile([P, HL * S], BF16, name=f"kt{b}", tag="kt")
        vsb = sbp.tile([P, NKB * LF], BF16, name=f"v{b}", tag="v")
        ot = sbp.tile([P, HL * S], BF16, name=f"ot{b}", tag="ot")

        for cb in range(NCB):
            # ---------------- x loads ----------------
            xtg = []
            for g in range(4):
                xt = sbp.tile([P, 4 * 512], BF16, name=f"xt{b}_{cb}_{g}",
                              tag="xt", bufs=8)
                eng = nc.sync if g % 2 == 0 else nc.scalar
                src = xT[g * 4 * P:(g + 1) * 4 * P,
                         b * S + cb * 512: b * S + (cb + 1) * 512]
                eng.dma_start(
                    out=xt,
                    in_=src.rearrange("(kc p) s -> p kc s", p=P))
                xtg.append(xt)
            xts = [xtg[kc // 4][:, (kc % 4) * 512:(kc % 4 + 1) * 512]
                   for kc in range(KC)]

            # ---------------- Q/K projections ----------------
            pqs = {}
            for key in ("q", "k"):
                for h in range(HL):
                    pqs[(key, h)] = psp.tile(
                        [P, 512], F32, name=f"p{key}{h}_{b}_{cb}",
                        tag=("po" if key == "q" else "pd"), bufs=2)
            for kc in range(KC):
                for key in ("q", "k"):
                    wsb = wq if key == "q" else wk
                    for h in range(HL):
                        nc.tensor.matmul(
                            pqs[(key, h)],
                            lhsT=wsb[:, kc * LF + h * DH: kc * LF + (h + 1) * DH],
                            rhs=xts[kc],
                            start=(kc == 0), stop=(kc == KC - 1))
            # RoPE drain: dst = pq*cos + rotate_half(pq)*sin (sin pre-signed)
            for key in ("q", "k"):
                dst = qt if key == "q" else kt
                for h in range(HL):
                    pq = pqs[(key, h)]
                    dsl = dst[:, h * S + cb * 512: h * S + (cb + 1) * 512]
                    cs = slice(cb * 512, (cb + 1) * 512)
                    ra = sbp.tile([P, 512], F32, name=f"ra{b}_{cb}_{key}{h}",
                                  tag="ex", bufs=4)
                    nc.vector.tensor_mul(ra, pq, cos[:, cs])
                    nc.vector.tensor_mul(dsl[0:64, :], pq[64:128, :],
                                         sin[0:64, cs])
                    nc.vector.tensor_mul(dsl[64:128, :], pq[0:64, :],
                                         sin[64:128, cs])
                    nc.vector.tensor_add(dsl, dsl, ra)

            # ---------------- V projection (natural layout) ----------------
            pvs = [psp.tile([P, LF], F32, name=f"pv{b}_{cb}_{r}", tag="ps",
                            bufs=4)
                   for r in range(4)]
            for kc in range(KC):
                for r in range(4):
                    nc.tensor.matmul(
                        pvs[r],
                        lhsT=xts[kc][:, r * P:(r + 1) * P],
                        rhs=wv[:, kc * LF:(kc + 1) * LF],
                        start=(kc == 0), stop=(kc == KC - 1))
            for r in range(4):
                kb = cb * 4 + r
                nc.scalar.copy(vsb[:, kb * LF:(kb + 1) * LF], pvs[r])

            # attention + partial output projection for this query block
            emit_attn(b, cb, qt, kt, vsb, ot)
            emit_wo(b, cb, ot)


def _make_emit_attn(nc, sbp, psp, band, ones):
    def emit_attn(b, qj, qt, kt, vsb, ot):
        # Both heads interleaved: each head's exp latency hides behind the
        # other head's matmuls.
        nkb = 4 * qj + 4
        po = {}
        pdn = {}
        for h in range(HL):
            po[h] = psp.tile([P, 512], F32, name=f"po{b}_{h}_{qj}",
                             tag="po", bufs=2)
            pdn[h] = psp.tile([1, 512], F32, name=f"pd{b}_{h}_{qj}",
                              tag="pd", bufs=2)
        exs = {}

        # For diagonal block r (kb = 4*qj + r), query columns [0, r*128) see
        # only masked keys in this block: skip them entirely — the scores
        # matmul, exp, attn@V and denominator all run on cols [r*128, 512).
        # The kb==0 matmuls always cover the full range (off=0 there), so
        # the accumulation start clears the whole bank.
        def _off(kb):
            return max(0, kb - 4 * qj) * P

        def emit_sc(h, kb):
            off = _off(kb)
            pss = psp.tile([P, 512], F32, name=f"pss{b}_{h}_{qj}_{kb}",
                           tag="ps", bufs=4)
            nc.tensor.matmul(
                pss[:, off:512],
                lhsT=kt[:, h * S + kb * P: h * S + (kb + 1) * P],
                rhs=qt[:, h * S + qj * 512 + off: h * S + (qj + 1) * 512],
                start=True, stop=True)
            ex = sbp.tile([P, 512], BF16, name=f"ex{b}_{h}_{qj}_{kb}",
                          tag="ex", bufs=4)
            nc.scalar.activation(ex[:, off:512], pss[:, off:512], AF.Exp,
                                 scale=SCALE)
            if kb >= 4 * qj:
                # upper-triangle mask on the diagonal 128-block
                nc.vector.tensor_mul(
                    ex[:, off:off + P], ex[:, off:off + P],
                    band[:, :])
            exs[(h, kb)] = ex

        def emit_av(h, kb, last):
            off = _off(kb)
            nc.tensor.matmul(
                po[h][:, off:512],
                lhsT=vsb[:, kb * LF + h * DH: kb * LF + h * DH + DH],
                rhs=exs[(h, kb)][:, off:512], start=(kb == 0), stop=last)
            nc.tensor.matmul(
                pdn[h][:, off:512], lhsT=ones,
                rhs=exs[(h, kb)][:, off:512], start=(kb == 0), stop=last)

        emit_sc(0, 0)
        emit_sc(1, 0)
        for kb in range(nkb):
            for h in range(HL):
                if kb + 1 < nkb:
                    emit_sc(h, kb + 1)
                emit_av(h, kb, last=(kb == nkb - 1))

        for h in range(HL):
            recip = sbp.tile([1, 512], F32, name=f"rc{b}_{h}_{qj}",
                             tag="rc", bufs=1)
            nc.vector.reciprocal(recip, pdn[h])
            bc = sbp.tile([P, 512], F32, name=f"bc{b}_{h}_{qj}",
                          tag="bc", bufs=1)
            nc.gpsimd.partition_broadcast(bc, recip)
            nc.vector.tensor_mul(
                ot[:, h * S + qj * 512: h * S + (qj + 1) * 512], po[h], bc)
    return emit_attn


def _make_emit_wo(nc, sbp, psp, wo, out):
    def emit_wo(b, qj, ot):
        for qc in range(4 * qj, 4 * qj + 4):
            st = None
            for nt in range(NNT):
                pw = psp.tile([P, 512], F32, name=f"pw{b}_{qc}_{nt}",
                              tag="ps", bufs=4)
                for h in range(HL):
                    nc.tensor.matmul(
                        pw,
                        lhsT=ot[:, h * S + qc * P: h * S + (qc + 1) * P],
                        rhs=wo[:, h * D + nt * 512: h * D + (nt + 1) * 512],
                        start=(h == 0), stop=(h == HL - 1))
                if nt % 2 == 0:
                    st = sbp.tile([P, 1024], BF16, name=f"st{b}_{qc}_{nt}",
                                  tag="st", bufs=8)
                    nc.scalar.copy(st[:, 0:512], pw)
                else:
                    nc.vector.tensor_copy(st[:, 512:1024], pw)
                    nc.sync.dma_start(
                        out=out[b * S + qc * P: b * S + (qc + 1) * P,
                                (nt - 1) * 512:(nt + 1) * 512],
                        in_=st)
    return emit_wo


def _build(loop_n=0):
    nc = bacc.Bacc("TRN2", target_bir_lowering=False, debug=False)
    t = {}
    t["xT"] = nc.dram_tensor("xT", [D, ROWS], BF16, kind="ExternalInput")
    t["wqT"] = nc.dram_tensor("wqT", [D, LF], BF16, kind="ExternalInput")
    t["wkT"] = nc.dram_tensor("wkT", [D, LF], BF16, kind="ExternalInput")
    t["wvT"] = nc.dram_tensor("wvT", [D, LF], BF16, kind="ExternalInput")
    t["woT"] = nc.dram_tensor("woT", [LF, D], BF16, kind="ExternalInput")
    t["cosT"] = nc.dram_tensor("cosT", [DH, S], F32, kind="ExternalInput")
    t["sinT"] = nc.dram_tensor("sinT", [DH, S], F32, kind="ExternalInput")
    t["bandT"] = nc.dram_tensor("bandT", [P, P], BF16, kind="ExternalInput")
    t["onesT"] = nc.dram_tensor("onesT", [P, 1], BF16, kind="ExternalInput")
    t["out"] = nc.dram_tensor("out", [ROWS, D], BF16, kind="ExternalOutput")
    with tile.TileContext(nc) as tc:
        with tc.tile_pool(name="sb", bufs=1) as sbp, \
             tc.tile_pool(name="ps", bufs=4, space="PSUM") as psp:
            c = _emit_consts(nc, sbp, t)
            if loop_n:
                with tc.For_i(0, loop_n, 1,
                              hint_engines=(mybir.EngineType.PE,
                                            mybir.EngineType.Activation,
                                            mybir.EngineType.DVE)):
                    _emit(nc, sbp, psp, c, t)
            else:
                _emit(nc, sbp, psp, c, t)
    nc.compile()
    return nc


def _tables():
    half = np.arange(0, DH, 2, dtype=np.float32) / np.float32(DH)
    inv_freq = (np.float32(1.0) / (np.float32(10000.0) ** half)).astype(np.float32)
    pos = np.arange(S, dtype=np.float32)
    freqs = np.outer(pos, inv_freq).astype(np.float32)        # [S, 64]
    emb = np.concatenate([freqs, freqs], axis=1)              # [S, DH]
    cosT = np.ascontiguousarray(np.cos(emb).T).astype(np.float32)
    sinT = np.sin(emb).T.astype(np.float32).copy()
    sinT[0:DH // 2, :] *= np.float32(-1.0)                    # pre-signed
    sinT = np.ascontiguousarray(sinT)
    # band[kl, c] = 1 iff c >= kl: upper-triangle mask for diagonal blocks
    kl = np.arange(P)[:, None]
    c = np.arange(P)[None, :]
    bandT = (c >= kl).astype(ml_dtypes.bfloat16)
    onesT = np.ones((P, 1), ml_dtypes.bfloat16)
    return cosT, sinT, bandT, onesT


def _in_maps(inputs):
    bf = ml_dtypes.bfloat16
    q = np.asarray(inputs["query"], dtype=np.float32)
    Wq = np.asarray(inputs["Wq"], dtype=np.float32)
    Wk = np.asarray(inputs["Wk"], dtype=np.float32)
    Wv = np.asarray(inputs["Wv"], dtype=np.float32)
    Wo = np.asarray(inputs["Wo"], dtype=np.float32)
    xT = np.ascontiguousarray(q.reshape(ROWS, D).T.astype(bf))
    cosT, sinT, bandT, onesT = _tables()
    maps = []
    for ci in range(NCORES):
        rs = slice(ci * LF, (ci + 1) * LF)
        maps.append({
            "xT": xT,
            "wqT": np.ascontiguousarray(Wq[rs, :].T.astype(bf)),
            "wkT": np.ascontiguousarray(Wk[rs, :].T.astype(bf)),
            "wvT": np.ascontiguousarray(Wv[rs, :].T.astype(bf)),
            "woT": np.ascontiguousarray(Wo[:, rs].T.astype(bf)),
            "cosT": cosT, "sinT": sinT, "bandT": bandT, "onesT": onesT,
        })
    return maps


def _run(inputs, trace=False, **kw):
    global _PROG
    if _PROG is None:
        _PROG = _build()
    res = run_bass_kernel_spmd(_PROG, _in_maps(inputs),
                               core_ids=list(range(NCORES)),
                               trace=trace, **kw)
    acc = np.zeros((ROWS, D), np.float64)
    for r in res.results:
        acc += r["out"].astype(np.float64)
    return acc.astype(np.float32).reshape(B, S, D), res


def kernel(query, Wq, Wk, Wv, Wo):
    out, _ = _run(dict(query=query, Wq=Wq, Wk=Wk, Wv=Wv, Wo=Wo))
    return out



# revision 6
# speedup vs baseline: 1.1949x; 1.1949x over previous
"""Causal attention with RoPE on 8 Trainium2 NeuronCores.

Tensor-parallel over heads: core c owns heads [2c, 2c+2). Each core computes
its heads' Q/K/V projections, RoPE, causal attention in a transposed layout
(keys on partitions), and a partial output projection through its slice of
Wo. The 8 partial outputs are summed on the host.

v2 schedule: the emission is software-pipelined so the in-order PE never
waits on cross-engine latency:
  - Projection matmuls of block i+1 are woven as filler between the
    dependent attention matmuls (scores -> exp -> attn@V) of block i.
  - The causal mask is applied on the PE itself: a small matmul writes
    -1e9 into the upper-triangle strip of the scores PSUM accumulation
    group before the QK matmul, so exp produces exact zeros and no
    vector-engine op sits between exp and attn@V.
  - Softmax denominators come from bf16 exp-sum accumulation on the DVE
    (E += ex per key block) followed by one ones^T @ E matmul per query
    block, replacing the per-block M=1 denominator matmuls on the PE.
  - Q/K projections run as per-(tensor, head) accumulation chains so RoPE
    can start draining each chain while the next accumulates.
PSUM: 2 banks QK ping/pong, 2 banks scores (+denominator), 4 banks shared
ring for attention-out / V projection / Wo tiles.
"""
import numpy as np
import ml_dtypes

import concourse.bacc as bacc
import concourse.bass as bass
import concourse.tile as tile
import concourse.mybir as mybir
from concourse.bass_utils import run_bass_kernel_spmd

AF = mybir.ActivationFunctionType
F32 = mybir.dt.float32
BF16 = mybir.dt.bfloat16

P = 128            # partitions
DH = 128           # head dim
D = 2048           # d_model
S = 2048           # sequence length
B = 2              # batch
NCORES = 8
HL = 2             # heads per core
LF = HL * DH       # 256 local head features
KC = D // P        # 16 d_model chunks
NCB = S // 512     # 4 column blocks of 512 positions per batch
NNT = D // 512     # 4 output column tiles
ROWS = B * S
SCALE = float(1.0 / np.sqrt(DH))

_PROG = None


def _emit_consts(nc, sbp, t):
    wq = sbp.tile([P, KC * LF], BF16, name="wq")
    wk = sbp.tile([P, KC * LF], BF16, name="wk")
    wv = sbp.tile([P, KC * LF], BF16, name="wv")
    wo = sbp.tile([P, HL * D], BF16, name="wo")
    cos = sbp.tile([DH, S], F32, name="cos")
    sin = sbp.tile([DH, S], F32, name="sin")
    ident = sbp.tile([P, P], BF16, name="ident")
    masku = sbp.tile([P, 4 * P], BF16, name="masku")
    ones = sbp.tile([P, 1], BF16, name="ones")
    for g in range(4):
        gk = slice(g * 4 * P, (g + 1) * 4 * P)
        nc.sync.dma_start(
            out=wq[:, g * 4 * LF:(g + 1) * 4 * LF],
            in_=t["wqT"][gk, :].rearrange("(kc p) f -> p kc f", p=P))
        nc.scalar.dma_start(
            out=wk[:, g * 4 * LF:(g + 1) * 4 * LF],
            in_=t["wkT"][gk, :].rearrange("(kc p) f -> p kc f", p=P))
        nc.gpsimd.dma_start(
            out=wv[:, g * 4 * LF:(g + 1) * 4 * LF],
            in_=t["wvT"][gk, :].rearrange("(kc p) f -> p kc f", p=P))
    nc.scalar.dma_start(out=cos, in_=t["cosT"][:, :])
    nc.scalar.dma_start(out=sin, in_=t["sinT"][:, :])
    nc.gpsimd.dma_start(out=ident, in_=t["identT"][:, :])
    nc.gpsimd.dma_start(out=masku, in_=t["maskuT"][:, :])
    nc.gpsimd.dma_start(out=ones, in_=t["onesT"][:, :])
    for h in range(HL):
        nc.gpsimd.dma_start(
            out=wo[:, h * D:(h + 1) * D],
            in_=t["woT"][h * P:(h + 1) * P, :])
    return dict(wq=wq, wk=wk, wv=wv, wo=wo, cos=cos, sin=sin, ident=ident,
                masku=masku, ones=ones)


class _Filler:
    """Round-robin distributor of independent PE work units (closures)."""

    def __init__(self):
        self.units = []
        self.total_w = 1
        self.acc = 0.0
        self.emitted = 0

    def load(self, units, total_w):
        self.units = list(units)
        self.total_w = max(1, total_w)
        self.acc = 0.0
        self.emitted = 0
        self.n0 = len(self.units)

    def take(self, w):
        self.acc += self.n0 * (w / self.total_w)
        while self.units and self.emitted < self.acc:
            self.units.pop(0)()
            self.emitted += 1

    def drain(self):
        while self.units:
            self.units.pop(0)()


def _emit(nc, sbp, psp, c, t):
    xT, out = t["xT"], t["out"]
    wq, wk, wv, wo = c["wq"], c["wk"], c["wv"], c["wo"]
    cos, sin = c["cos"], c["sin"]
    ident, masku, ones = c["ident"], c["masku"], c["ones"]

    iters = [(b, cb) for b in range(B) for cb in range(NCB)]
    state = {"xt": {}, "qt": {}, "kt": {}, "vsb": {}}

    def emit_xload(i):
        b, cb = iters[i]
        xtg = []
        for g in range(4):
            xt = sbp.tile([P, 4 * 512], BF16, name=f"xt{b}_{cb}_{g}",
                          tag="xt", bufs=8)
            eng = nc.sync if g % 2 == 0 else nc.gpsimd
            src = xT[g * 4 * P:(g + 1) * 4 * P,
                     b * S + cb * 512: b * S + (cb + 1) * 512]
            eng.dma_start(out=xt, in_=src.rearrange("(kc p) s -> p kc s", p=P))
            xtg.append(xt)
        state["xt"][i] = [xtg[kc // 4][:, (kc % 4) * 512:(kc % 4 + 1) * 512]
                         for kc in range(KC)]

    def build_proj_units(i):
        """48 closures: 32 QK units (2 matmuls each, RoPE on chain end),
        16 V units (2 matmuls each, drain on sub-chain end)."""
        b, cb = iters[i]
        xts = state["xt"][i]
        qt = sbp.tile([P, HL * 512], BF16, name=f"qt{b}_{cb}", tag="qt",
                      bufs=2)
        state["qt"][i] = qt
        if cb == 0:
            state["kt"][b] = sbp.tile([P, HL * S], BF16, name=f"kt{b}",
                                      tag="kt", bufs=2)
            state["vsb"][b] = sbp.tile([P, 4 * NCB * LF], BF16,
                                       name=f"v{b}", tag="vsb", bufs=2)
        kt, vsb = state["kt"][b], state["vsb"][b]
        units = []
        cs = slice(cb * 512, (cb + 1) * 512)

        def mk_rope(pq, dst):
            def rope():
                ra = sbp.tile([P, 512], F32, name="ra", tag="ra", bufs=2)
                nc.vector.tensor_mul(ra, pq, cos[:, cs])
                nc.vector.tensor_mul(dst[0:64, :], pq[64:128, :],
                                     sin[0:64, cs])
                nc.vector.tensor_mul(dst[64:128, :], pq[0:64, :],
                                     sin[64:128, cs])
                nc.vector.tensor_add(dst, dst, ra)
            return rope

        for key in ("q", "k"):
            wsb = wq if key == "q" else wk
            for h in range(HL):
                pq = psp.tile([P, 512], F32, name=f"p{key}{h}_{b}_{cb}",
                              tag="qk", bufs=2)
                if key == "q":
                    dst = qt[:, h * 512:(h + 1) * 512]
                else:
                    dst = kt[:, h * S + cb * 512: h * S + (cb + 1) * 512]
                rope = mk_rope(pq, dst)
                for j in range(8):
                    def u(j=j, pq=pq, wsb=wsb, h=h, rope=rope):
                        for kc in (2 * j, 2 * j + 1):
                            nc.tensor.matmul(
                                pq,
                                lhsT=wsb[:, kc * LF + h * DH:
                                         kc * LF + (h + 1) * DH],
                                rhs=xts[kc],
                                start=(kc == 0), stop=(kc == KC - 1))
                        if j == 7:
                            rope()
                    units.append(u)

        for pair in range(2):
            pv = psp.tile([P, 512], F32, name=f"pv{b}_{cb}_{pair}",
                          tag="acc", bufs=4)

            def mk_vdrain(pv=pv, pair=pair):
                def vdrain():
                    kb0 = cb * 4 + pair * 2
                    nc.scalar.copy(
                        vsb[:, kb0 * LF:(kb0 + 2) * LF], pv)
                return vdrain
            vdrain = mk_vdrain()
            for r in (0, 1):
                for j in range(8):
                    def u(j=j, pv=pv, r=r, pair=pair, vdrain=vdrain):
                        gr = pair * 2 + r
                        for kc in (2 * j, 2 * j + 1):
                            nc.tensor.matmul(
                                pv[:, r * 256:(r + 1) * 256],
                                lhsT=xts[kc][:, gr * P:(gr + 1) * P],
                                rhs=wv[:, kc * LF:(kc + 1) * LF],
                                start=(kc == 0), stop=(kc == KC - 1))
                        if r == 1 and j == 7:
                            vdrain()
                    units.append(u)
        return units

    def emit_attention(i, fill):
        """Attention + tail + Wo for iteration i, weaving filler units."""
        b, qj = iters[i]
        nkb = 4 * qj + 4
        qt = state["qt"][i]
        kt, vsb = state["kt"][b], state["vsb"][b]

        po = [psp.tile([P, 512], F32, name=f"po{b}_{h}_{qj}", tag="acc",
                       bufs=4) for h in range(HL)]
        E = sbp.tile([P, HL * 512], BF16, name=f"E{b}_{qj}", tag="E", bufs=2)
        exs = {}

        def _off(kb):
            return max(0, kb - 4 * qj) * P

        def ph1(kb):
            # scores (with PE-side causal mask on the diagonal strip) + exp
            off = _off(kb)
            diag = kb >= 4 * qj
            pss = {}
            for h in range(HL):
                pss[h] = psp.tile([P, 512], F32, name=f"pss{b}_{h}_{qj}_{kb}",
                                  tag="ps", bufs=2)
                lk = kt[:, h * S + kb * P: h * S + (kb + 1) * P]
                if diag:
                    # -1e9 upper-triangle strip (zeros beyond), then
                    # accumulate k^T q on top: exp gives exact 0s, no
                    # vector-engine op between exp and attn@V
                    nc.tensor.matmul(
                        pss[h][:, off:512], lhsT=ident,
                        rhs=masku[:, 0:512 - off],
                        start=True, stop=False)
                    nc.tensor.matmul(
                        pss[h][:, off:512], lhsT=lk,
                        rhs=qt[:, h * 512 + off:(h + 1) * 512],
                        start=False, stop=True)
                else:
                    nc.tensor.matmul(
                        pss[h], lhsT=lk, rhs=qt[:, h * 512:(h + 1) * 512],
                        start=True, stop=True)
            ex = sbp.tile([P, HL * 512], BF16, name=f"ex{b}_{qj}_{kb}",
                          tag="ex", bufs=4)
            for h in range(HL):
                nc.scalar.activation(ex[:, h * 512 + off:(h + 1) * 512],
                                     pss[h][:, off:512], AF.Exp, scale=SCALE)
            exs[kb] = ex

        def ph2(kb):
            # attn@V + denominator partial accumulation
            off = _off(kb)
            ex = exs[kb]
            last = kb == nkb - 1
            for h in range(HL):
                nc.tensor.matmul(
                    po[h][:, off:512],
                    lhsT=vsb[:, kb * LF + h * DH: kb * LF + (h + 1) * DH],
                    rhs=ex[:, h * 512 + off:(h + 1) * 512],
                    start=(kb == 0), stop=last)
            if kb == 0:
                nc.vector.tensor_copy(E, ex)
            else:
                nc.vector.tensor_add(E[:, off:512], E[:, off:512],
                                     ex[:, off:512])
                nc.vector.tensor_add(E[:, 512 + off:1024],
                                     E[:, 512 + off:1024],
                                     ex[:, 512 + off:1024])

        for kb in range(nkb):
            ph1(kb)
            fill.take(1)
            if kb >= 2:
                ph2(kb - 2)
        fill.take(1)
        ph2(nkb - 2)
        fill.take(1)
        ph2(nkb - 1)
        fill.take(2)

        # tail: denominators -> reciprocal -> broadcast -> normalize
        pdd = [psp.tile([1, 512], F32, name=f"pd{b}_{h}_{qj}", tag="ps",
                        bufs=2) for h in range(HL)]
        for h in range(HL):
            nc.tensor.matmul(pdd[h], lhsT=ones,
                             rhs=E[:, h * 512:(h + 1) * 512],
                             start=True, stop=True)
        ot = sbp.tile([P, HL * 512], BF16, name=f"ot{b}_{qj}", tag="ot",
                      bufs=2)
        for h in range(HL):
            recip = sbp.tile([1, 512], F32, name="rc", tag="rc", bufs=2)
            nc.vector.reciprocal(recip, pdd[h])
            bc = sbp.tile([P, 512], F32, name="bc", tag="bc", bufs=2)
            nc.gpsimd.partition_broadcast(bc, recip)
            nc.vector.tensor_mul(ot[:, h * 512:(h + 1) * 512], po[h], bc)
        # all remaining filler covers the normalize latency and guarantees
        # the woven V-projection drains land before the Wo tiles reuse
        # their PSUM ring slots
        fill.drain()

        # Wo: per 128-row chunk qc, accumulate both heads into pw per nt
        for qc in range(4 * qj, 4 * qj + 4):
            st = None
            for nt in range(NNT):
                pw = psp.tile([P, 512], F32, name=f"pw{b}_{qc}_{nt}",
                              tag="acc", bufs=4)
                for h in range(HL):
                    nc.tensor.matmul(
                        pw,
                        lhsT=ot[:, h * 512 + (qc - 4 * qj) * P:
                                h * 512 + (qc - 4 * qj + 1) * P],
                        rhs=wo[:, h * D + nt * 512: h * D + (nt + 1) * 512],
                        start=(h == 0), stop=(h == HL - 1))
                if nt % 2 == 0:
                    st = sbp.tile([P, 1024], BF16, name=f"st{b}_{qc}_{nt}",
                                  tag="st", bufs=8)
                    nc.scalar.copy(st[:, 0:512], pw)
                else:
                    nc.vector.tensor_copy(st[:, 512:1024], pw)
                    nc.sync.dma_start(
                        out=out[b * S + qc * P: b * S + (qc + 1) * P,
                                (nt - 1) * 512:(nt + 1) * 512],
                        in_=st)

    # ---- prologue: x loads for iterations 0,1; projections for 0 ----
    emit_xload(0)
    emit_xload(1)
    fill = _Filler()
    for u in build_proj_units(0):
        u()
    n = len(iters)
    for i in range(n):
        if i + 2 < n:
            emit_xload(i + 2)
        if i + 1 < n:
            units = build_proj_units(i + 1)
        else:
            units = []
        b, qj = iters[i]
        nkb = 4 * qj + 4
        total_w = nkb + 4
        fill.load(units, total_w)
        emit_attention(i, fill)
        fill.drain()


def _build(loop_n=0):
    nc = bacc.Bacc("TRN2", target_bir_lowering=False, debug=False)
    t = {}
    t["xT"] = nc.dram_tensor("xT", [D, ROWS], BF16, kind="ExternalInput")
    t["wqT"] = nc.dram_tensor("wqT", [D, LF], BF16, kind="ExternalInput")
    t["wkT"] = nc.dram_tensor("wkT", [D, LF], BF16, kind="ExternalInput")
    t["wvT"] = nc.dram_tensor("wvT", [D, LF], BF16, kind="ExternalInput")
    t["woT"] = nc.dram_tensor("woT", [LF, D], BF16, kind="ExternalInput")
    t["cosT"] = nc.dram_tensor("cosT", [DH, S], F32, kind="ExternalInput")
    t["sinT"] = nc.dram_tensor("sinT", [DH, S], F32, kind="ExternalInput")
    t["identT"] = nc.dram_tensor("identT", [P, P], BF16, kind="ExternalInput")
    t["maskuT"] = nc.dram_tensor("maskuT", [P, 4 * P], BF16, kind="ExternalInput")
    t["onesT"] = nc.dram_tensor("onesT", [P, 1], BF16, kind="ExternalInput")
    t["out"] = nc.dram_tensor("out", [ROWS, D], BF16, kind="ExternalOutput")
    with tile.TileContext(nc) as tc:
        with tc.tile_pool(name="sb", bufs=1) as sbp, \
             tc.tile_pool(name="ps", bufs=2, space="PSUM") as psp:
            c = _emit_consts(nc, sbp, t)
            if loop_n:
                with tc.For_i(0, loop_n, 1,
                              hint_engines=(mybir.EngineType.PE,
                                            mybir.EngineType.Activation,
                                            mybir.EngineType.DVE)):
                    _emit(nc, sbp, psp, c, t)
            else:
                _emit(nc, sbp, psp, c, t)
    nc.compile()
    return nc


def _tables():
    half = np.arange(0, DH, 2, dtype=np.float32) / np.float32(DH)
    inv_freq = (np.float32(1.0) / (np.float32(10000.0) ** half)).astype(np.float32)
    pos = np.arange(S, dtype=np.float32)
    freqs = np.outer(pos, inv_freq).astype(np.float32)        # [S, 64]
    emb = np.concatenate([freqs, freqs], axis=1)              # [S, DH]
    cosT = np.ascontiguousarray(np.cos(emb).T).astype(np.float32)
    sinT = np.sin(emb).T.astype(np.float32).copy()
    sinT[0:DH // 2, :] *= np.float32(-1.0)                    # pre-signed
    sinT = np.ascontiguousarray(sinT)
    identT = np.eye(P, dtype=ml_dtypes.bfloat16)
    # masku[k, c] = -1e9 iff c < k (query c attends only to keys <= c)
    kl = np.arange(P)[:, None]
    cc = np.arange(P)[None, :]
    masku_core = np.where(cc < kl, np.float32(-1e9),
                          np.float32(0.0)).astype(ml_dtypes.bfloat16)
    maskuT = np.zeros((P, 4 * P), ml_dtypes.bfloat16)
    maskuT[:, 0:P] = masku_core
    onesT = np.ones((P, 1), ml_dtypes.bfloat16)
    return cosT, sinT, identT, maskuT, onesT


def _in_maps(inputs):
    bf = ml_dtypes.bfloat16
    q = np.asarray(inputs["query"], dtype=np.float32)
    Wq = np.asarray(inputs["Wq"], dtype=np.float32)
    Wk = np.asarray(inputs["Wk"], dtype=np.float32)
    Wv = np.asarray(inputs["Wv"], dtype=np.float32)
    Wo = np.asarray(inputs["Wo"], dtype=np.float32)
    xT = np.ascontiguousarray(q.reshape(ROWS, D).T.astype(bf))
    cosT, sinT, identT, maskuT, onesT = _tables()
    maps = []
    for ci in range(NCORES):
        rs = slice(ci * LF, (ci + 1) * LF)
        maps.append({
            "xT": xT,
            "wqT": np.ascontiguousarray(Wq[rs, :].T.astype(bf)),
            "wkT": np.ascontiguousarray(Wk[rs, :].T.astype(bf)),
            "wvT": np.ascontiguousarray(Wv[rs, :].T.astype(bf)),
            "woT": np.ascontiguousarray(Wo[:, rs].T.astype(bf)),
            "cosT": cosT, "sinT": sinT, "identT": identT,
            "maskuT": maskuT, "onesT": onesT,
        })
    return maps


def _run(inputs, trace=False, **kw):
    global _PROG
    if _PROG is None:
        _PROG = _build()
    res = run_bass_kernel_spmd(_PROG, _in_maps(inputs),
                               core_ids=list(range(NCORES)),
                               trace=trace, **kw)
    acc = np.zeros((ROWS, D), np.float64)
    for r in res.results:
        acc += r["out"].astype(np.float64)
    return acc.astype(np.float32).reshape(B, S, D), res


def kernel(query, Wq, Wk, Wv, Wo):
    out, _ = _run(dict(query=query, Wq=Wq, Wk=Wk, Wv=Wv, Wo=Wo))
    return out


# revision 14
# speedup vs baseline: 1.2043x; 1.0078x over previous
"""Causal attention with RoPE on 8 Trainium2 NeuronCores.

Tensor-parallel over heads: core c owns heads [2c, 2c+2). Each core computes
its heads' Q/K/V projections, RoPE, causal attention in a transposed layout
(keys on partitions), and a partial output projection through its slice of
Wo. The 8 partial outputs are summed on the host.

v2 schedule: the emission is software-pipelined so the in-order PE never
waits on cross-engine latency:
  - Projection matmuls of block i+1 are woven as filler between the
    dependent attention matmuls (scores -> exp -> attn@V) of block i.
  - The causal mask is applied on the PE itself: a small matmul writes
    -1e9 into the upper-triangle strip of the scores PSUM accumulation
    group before the QK matmul, so exp produces exact zeros and no
    vector-engine op sits between exp and attn@V.
  - Softmax denominators come from bf16 exp-sum accumulation on the DVE
    (E += ex per key block) followed by one ones^T @ E matmul per query
    block, replacing the per-block M=1 denominator matmuls on the PE.
  - Q/K projections run as per-(tensor, head) accumulation chains so RoPE
    can start draining each chain while the next accumulates.
PSUM: 2 banks QK ping/pong, 2 banks scores (+denominator), 4 banks shared
ring for attention-out / V projection / Wo tiles.
"""
import numpy as np
import ml_dtypes

import concourse.bacc as bacc
import concourse.bass as bass
import concourse.tile as tile
import concourse.mybir as mybir
from concourse.bass_utils import run_bass_kernel_spmd

AF = mybir.ActivationFunctionType
F32 = mybir.dt.float32
BF16 = mybir.dt.bfloat16

P = 128            # partitions
DH = 128           # head dim
D = 2048           # d_model
S = 2048           # sequence length
B = 2              # batch
NCORES = 8
HL = 2             # heads per core
LF = HL * DH       # 256 local head features
KC = D // P        # 16 d_model chunks
NCB = S // 512     # 4 column blocks of 512 positions per batch
NNT = D // 512     # 4 output column tiles
ROWS = B * S
SCALE = float(1.0 / np.sqrt(DH))

_PROG = None
VARIANT = "base"


def _emit_consts(nc, sbp, t):
    wq = sbp.tile([P, KC * LF], BF16, name="wq")
    wk = sbp.tile([P, KC * LF], BF16, name="wk")
    wv = sbp.tile([P, KC * LF], BF16, name="wv")
    wo = sbp.tile([P, HL * D], BF16, name="wo")
    cos = sbp.tile([DH, S], F32, name="cos")
    sin = sbp.tile([DH, S], F32, name="sin")
    ident = sbp.tile([P, P], BF16, name="ident")
    masku = sbp.tile([P, 4 * P], BF16, name="masku")
    ones = sbp.tile([P, 1], BF16, name="ones")
    for g in range(4):
        gk = slice(g * 4 * P, (g + 1) * 4 * P)
        nc.sync.dma_start(
            out=wq[:, g * 4 * LF:(g + 1) * 4 * LF],
            in_=t["wqT"][gk, :].rearrange("(kc p) f -> p kc f", p=P))
        nc.scalar.dma_start(
            out=wk[:, g * 4 * LF:(g + 1) * 4 * LF],
            in_=t["wkT"][gk, :].rearrange("(kc p) f -> p kc f", p=P))
        nc.gpsimd.dma_start(
            out=wv[:, g * 4 * LF:(g + 1) * 4 * LF],
            in_=t["wvT"][gk, :].rearrange("(kc p) f -> p kc f", p=P))
    nc.scalar.dma_start(out=cos, in_=t["cosT"][:, :])
    nc.scalar.dma_start(out=sin, in_=t["sinT"][:, :])
    nc.gpsimd.dma_start(out=ident, in_=t["identT"][:, :])
    nc.gpsimd.dma_start(out=masku, in_=t["maskuT"][:, :])
    nc.gpsimd.dma_start(out=ones, in_=t["onesT"][:, :])
    for h in range(HL):
        nc.gpsimd.dma_start(
            out=wo[:, h * D:(h + 1) * D],
            in_=t["woT"][h * P:(h + 1) * P, :])
    return dict(wq=wq, wk=wk, wv=wv, wo=wo, cos=cos, sin=sin, ident=ident,
                masku=masku, ones=ones)


class _Filler:
    """Round-robin distributor of independent PE work units (closures)."""

    def __init__(self):
        self.units = []
        self.total_w = 1
        self.acc = 0.0
        self.emitted = 0

    def load(self, units, total_w):
        self.units = list(units)
        self.total_w = max(1, total_w)
        self.acc = 0.0
        self.emitted = 0
        self.n0 = len(self.units)

    def take(self, w):
        self.acc += self.n0 * (w / self.total_w)
        while self.units and self.emitted < self.acc:
            self.units.pop(0)()
            self.emitted += 1

    def drain(self):
        while self.units:
            self.units.pop(0)()


def _emit(nc, sbp, psp, c, t):
    xT, out = t["xT"], t["out"]
    wq, wk, wv, wo = c["wq"], c["wk"], c["wv"], c["wo"]
    cos, sin = c["cos"], c["sin"]
    ident, masku, ones = c["ident"], c["masku"], c["ones"]

    iters = [(b, cb) for b in range(B) for cb in range(NCB)]
    state = {"xt": {}, "qt": {}, "kt": {}, "vsb": {}}

    def emit_xload(i):
        if VARIANT == "noxload" and i >= 2:
            state["xt"][i] = state["xt"][i % 2]
            return
        b, cb = iters[i]
        xtg = []
        for g in range(4):
            xt = sbp.tile([P, 4 * 512], BF16, name=f"xt{b}_{cb}_{g}",
                          tag="xt", bufs=8)
            eng = nc.sync if g % 2 == 0 else nc.gpsimd
            src = xT[g * 4 * P:(g + 1) * 4 * P,
                     b * S + cb * 512: b * S + (cb + 1) * 512]
            eng.dma_start(out=xt, in_=src.rearrange("(kc p) s -> p kc s", p=P))
            xtg.append(xt)
        state["xt"][i] = [xtg[kc // 4][:, (kc % 4) * 512:(kc % 4 + 1) * 512]
                         for kc in range(KC)]

    def build_proj_units(i):
        """48 closures: 32 QK units (2 matmuls each, RoPE on chain end),
        16 V units (2 matmuls each, drain on sub-chain end)."""
        b, cb = iters[i]
        xts = state["xt"][i]
        qt = sbp.tile([P, HL * 512], BF16, name=f"qt{b}_{cb}", tag="qt",
                      bufs=2)
        state["qt"][i] = qt
        if cb == 0:
            state["kt"][b] = sbp.tile([P, HL * S], BF16, name=f"kt{b}",
                                      tag="kt", bufs=2)
            state["vsb"][b] = sbp.tile([P, 4 * NCB * LF], BF16,
                                       name=f"v{b}", tag="vsb", bufs=2)
        kt, vsb = state["kt"][b], state["vsb"][b]
        units = []
        cs = slice(cb * 512, (cb + 1) * 512)

        def mk_rope(pq, dst):
            def rope():
                if VARIANT == "norope":
                    nc.vector.tensor_copy(dst, pq)
                    return
                ra = sbp.tile([P, 512], F32, name="ra", tag="ra", bufs=2)
                nc.vector.tensor_mul(ra, pq, cos[:, cs])
                nc.vector.tensor_mul(dst[0:64, :], pq[64:128, :],
                                     sin[0:64, cs])
                nc.vector.tensor_mul(dst[64:128, :], pq[0:64, :],
                                     sin[64:128, cs])
                nc.vector.tensor_add(dst, dst, ra)
            return rope

        if VARIANT == "noproj":
            nc.sync.dma_start(out=qt[:, 0:512], in_=xts[0])
            nc.sync.dma_start(out=qt[:, 512:1024], in_=xts[1])
            for h in range(HL):
                nc.gpsimd.dma_start(
                    out=kt[:, h * S + cb * 512:h * S + (cb + 1) * 512],
                    in_=xts[2 + h])
            nc.sync.dma_start(out=vsb[:, cb * 4 * LF:cb * 4 * LF + 512],
                              in_=xts[4])
            nc.sync.dma_start(out=vsb[:, cb * 4 * LF + 512:(cb + 1) * 4 * LF],
                              in_=xts[5])
            return units
        for key in ("q", "k"):
            wsb = wq if key == "q" else wk
            for h in range(HL):
                pq = psp.tile([P, 512], F32, name=f"p{key}{h}_{b}_{cb}",
                              tag="qk", bufs=2)
                if key == "q":
                    dst = qt[:, h * 512:(h + 1) * 512]
                else:
                    dst = kt[:, h * S + cb * 512: h * S + (cb + 1) * 512]
                rope = mk_rope(pq, dst)
                for j in range(8):
                    def u(j=j, pq=pq, wsb=wsb, h=h, rope=rope):
                        for kc in (2 * j, 2 * j + 1):
                            nc.tensor.matmul(
                                pq,
                                lhsT=wsb[:, kc * LF + h * DH:
                                         kc * LF + (h + 1) * DH],
                                rhs=xts[kc],
                                start=(kc == 0), stop=(kc == KC - 1))
                        if j == 7:
                            rope()
                    units.append(u)

        for pair in range(2):
            pv = psp.tile([P, 512], F32, name=f"pv{b}_{cb}_{pair}",
                          tag="acc", bufs=4)

            def mk_vdrain(pv=pv, pair=pair):
                def vdrain():
                    kb0 = cb * 4 + pair * 2
                    nc.scalar.copy(
                        vsb[:, kb0 * LF:(kb0 + 2) * LF], pv)
                return vdrain
            vdrain = mk_vdrain()
            for r in (0, 1):
                for j in range(8):
                    def u(j=j, pv=pv, r=r, pair=pair, vdrain=vdrain):
                        gr = pair * 2 + r
                        for kc in (2 * j, 2 * j + 1):
                            nc.tensor.matmul(
                                pv[:, r * 256:(r + 1) * 256],
                                lhsT=xts[kc][:, gr * P:(gr + 1) * P],
                                rhs=wv[:, kc * LF:(kc + 1) * LF],
                                start=(kc == 0), stop=(kc == KC - 1))
                        if r == 1 and j == 7:
                            vdrain()
                    units.append(u)
        return units

    def emit_attention(i, fill):
        """Attention + tail + Wo for iteration i, weaving filler units."""
        b, qj = iters[i]
        nkb = 4 * qj + 4
        qt = state["qt"][i]
        kt, vsb = state["kt"][b], state["vsb"][b]

        po = [psp.tile([P, 512], F32, name=f"po{b}_{h}_{qj}", tag="acc",
                       bufs=4) for h in range(HL)]
        E = sbp.tile([P, HL * 512], BF16, name=f"E{b}_{qj}", tag="E", bufs=2)
        exs = {}

        def _off(kb):
            return max(0, kb - 4 * qj) * P

        def ph1(kb):
            # scores (with PE-side causal mask on the diagonal strip) + exp
            off = _off(kb)
            diag = kb >= 4 * qj
            pss = {}
            for h in range(HL):
                pss[h] = psp.tile([P, 512], F32, name=f"pss{b}_{h}_{qj}_{kb}",
                                  tag="ps", bufs=2)
                lk = kt[:, h * S + kb * P: h * S + (kb + 1) * P]
                if diag:
                    # -1e9 upper-triangle strip (zeros beyond), then
                    # accumulate k^T q on top: exp gives exact 0s, no
                    # vector-engine op between exp and attn@V
                    nc.tensor.matmul(
                        pss[h][:, off:512], lhsT=ident,
                        rhs=masku[:, 0:512 - off],
                        start=True, stop=False)
                    nc.tensor.matmul(
                        pss[h][:, off:512], lhsT=lk,
                        rhs=qt[:, h * 512 + off:(h + 1) * 512],
                        start=False, stop=True)
                else:
                    nc.tensor.matmul(
                        pss[h], lhsT=lk, rhs=qt[:, h * 512:(h + 1) * 512],
                        start=True, stop=True)
            ex = sbp.tile([P, HL * 512], BF16, name=f"ex{b}_{qj}_{kb}",
                          tag="ex", bufs=4)
            fn = AF.Copy if VARIANT == "expcopy" else AF.Exp
            for h in range(HL):
                nc.scalar.activation(ex[:, h * 512 + off:(h + 1) * 512],
                                     pss[h][:, off:512], fn, scale=SCALE)
            exs[kb] = ex

        def ph2(kb):
            # attn@V + denominator partial accumulation
            off = _off(kb)
            ex = exs[kb]
            last = kb == nkb - 1
            for h in range(HL):
                nc.tensor.matmul(
                    po[h][:, off:512],
                    lhsT=vsb[:, kb * LF + h * DH: kb * LF + (h + 1) * DH],
                    rhs=ex[:, h * 512 + off:(h + 1) * 512],
                    start=(kb == 0), stop=last)
            if VARIANT == "noeadd":
                pass
            elif kb == 0:
                nc.vector.tensor_copy(E, ex)
            else:
                nc.vector.tensor_add(E[:, off:512], E[:, off:512],
                                     ex[:, off:512])
                nc.vector.tensor_add(E[:, 512 + off:1024],
                                     E[:, 512 + off:1024],
                                     ex[:, 512 + off:1024])

        lag = 3 if VARIANT == "lag3" else 2
        if VARIANT == "noattn":
            nkb = 0
        for kb in range(nkb):
            ph1(kb)
            fill.take(1)
            if kb >= lag:
                ph2(kb - lag)
        for r in range(lag, 0, -1):
            if nkb - r >= 0:
                fill.take(1)
                ph2(nkb - r)
        fill.take(2)

        # tail: denominators -> reciprocal -> broadcast -> normalize
        skip_tail = VARIANT == "noattn"
        pdd = [psp.tile([1, 512], F32, name=f"pd{b}_{h}_{qj}", tag="ps",
                        bufs=2) for h in range(HL)]
        for h in range(HL):
            if skip_tail:
                break
            nc.tensor.matmul(pdd[h], lhsT=ones,
                             rhs=E[:, h * 512:(h + 1) * 512],
                             start=True, stop=True)
        ot = sbp.tile([P, HL * 512], BF16, name=f"ot{b}_{qj}", tag="ot",
                      bufs=2)
        for h in range(HL):
            if skip_tail:
                break
            recip = sbp.tile([1, 512], F32, name="rc", tag="rc", bufs=2)
            if VARIANT == "noeadd":
                nc.vector.tensor_copy(ot[:, h * 512:(h + 1) * 512], po[h])
                continue
            nc.vector.reciprocal(recip, pdd[h])
            bc = sbp.tile([P, 512], F32, name="bc", tag="bc", bufs=2)
            nc.gpsimd.partition_broadcast(bc, recip)
            nc.vector.tensor_mul(ot[:, h * 512:(h + 1) * 512], po[h], bc)
        # all remaining filler covers the normalize latency and guarantees
        # the woven V-projection drains land before the Wo tiles reuse
        # their PSUM ring slots
        fill.drain()

        # Wo: per 128-row chunk qc, accumulate both heads into pw per nt.
        # Drains rotate across ACT/DVE/Pool so no single engine paces the
        # pw ring; DMA per 1024-col pair from bf16 staging
        for qc in range(4 * qj, 4 * qj + 4):
            st = None
            for nt in range(NNT):
                pw = psp.tile([P, 512], F32, name=f"pw{b}_{qc}_{nt}",
                              tag="acc", bufs=4)
                for h in range(HL):
                    src = vsb if VARIANT == "noattn" else ot
                    nc.tensor.matmul(
                        pw,
                        lhsT=src[:, h * 512 + (qc - 4 * qj) * P:
                                 h * 512 + (qc - 4 * qj + 1) * P],
                        rhs=wo[:, h * D + nt * 512: h * D + (nt + 1) * 512],
                        start=(h == 0), stop=(h == HL - 1))
                if nt % 2 == 0:
                    st = sbp.tile([P, 1024], BF16, name=f"st{b}_{qc}_{nt}",
                                  tag="st", bufs=8)
                dsl = st[:, (nt % 2) * 512:(nt % 2 + 1) * 512]
                if nt % 2 == 0:
                    nc.scalar.copy(dsl, pw)
                else:
                    nc.vector.tensor_copy(dsl, pw)
                if nt % 2 == 1 and VARIANT != "noout":
                    nc.sync.dma_start(
                        out=out[b * S + qc * P: b * S + (qc + 1) * P,
                                (nt - 1) * 512:(nt + 1) * 512],
                        in_=st)

    # ---- prologue: x loads for iterations 0,1; projections for 0 ----
    emit_xload(0)
    emit_xload(1)
    fill = _Filler()
    for u in build_proj_units(0):
        u()
    n = len(iters)
    for i in range(n):
        if i + 2 < n:
            emit_xload(i + 2)
        if i + 1 < n:
            units = build_proj_units(i + 1)
        else:
            units = []
        b, qj = iters[i]
        nkb = 4 * qj + 4
        total_w = nkb + 4
        fill.load(units, total_w)
        emit_attention(i, fill)
        fill.drain()


def _build(loop_n=0, nbody=1):
    nc = bacc.Bacc("TRN2", target_bir_lowering=False, debug=False)
    t = {}
    t["xT"] = nc.dram_tensor("xT", [D, ROWS], BF16, kind="ExternalInput")
    t["wqT"] = nc.dram_tensor("wqT", [D, LF], BF16, kind="ExternalInput")
    t["wkT"] = nc.dram_tensor("wkT", [D, LF], BF16, kind="ExternalInput")
    t["wvT"] = nc.dram_tensor("wvT", [D, LF], BF16, kind="ExternalInput")
    t["woT"] = nc.dram_tensor("woT", [LF, D], BF16, kind="ExternalInput")
    t["cosT"] = nc.dram_tensor("cosT", [DH, S], F32, kind="ExternalInput")
    t["sinT"] = nc.dram_tensor("sinT", [DH, S], F32, kind="ExternalInput")
    t["identT"] = nc.dram_tensor("identT", [P, P], BF16, kind="ExternalInput")
    t["maskuT"] = nc.dram_tensor("maskuT", [P, 4 * P], BF16, kind="ExternalInput")
    t["onesT"] = nc.dram_tensor("onesT", [P, 1], BF16, kind="ExternalInput")
    t["out"] = nc.dram_tensor("out", [ROWS, D], BF16, kind="ExternalOutput")
    with tile.TileContext(nc) as tc:
        with tc.tile_pool(name="sb", bufs=1) as sbp, \
             tc.tile_pool(name="ps", bufs=2, space="PSUM") as psp:
            c = _emit_consts(nc, sbp, t)
            if loop_n:
                with tc.For_i(0, loop_n, 1,
                              hint_engines=(mybir.EngineType.PE,
                                            mybir.EngineType.Activation,
                                            mybir.EngineType.DVE)):
                    for _ in range(nbody):
                        _emit(nc, sbp, psp, c, t)
            else:
                _emit(nc, sbp, psp, c, t)
    nc.compile()
    return nc


def _tables():
    half = np.arange(0, DH, 2, dtype=np.float32) / np.float32(DH)
    inv_freq = (np.float32(1.0) / (np.float32(10000.0) ** half)).astype(np.float32)
    pos = np.arange(S, dtype=np.float32)
    freqs = np.outer(pos, inv_freq).astype(np.float32)        # [S, 64]
    emb = np.concatenate([freqs, freqs], axis=1)              # [S, DH]
    cosT = np.ascontiguousarray(np.cos(emb).T).astype(np.float32)
    sinT = np.sin(emb).T.astype(np.float32).copy()
    sinT[0:DH // 2, :] *= np.float32(-1.0)                    # pre-signed
    sinT = np.ascontiguousarray(sinT)
    identT = np.eye(P, dtype=ml_dtypes.bfloat16)
    # masku[k, c] = -1e9 iff c < k (query c attends only to keys <= c)
    kl = np.arange(P)[:, None]
    cc = np.arange(P)[None, :]
    masku_core = np.where(cc < kl, np.float32(-1e9),
                          np.float32(0.0)).astype(ml_dtypes.bfloat16)
    maskuT = np.zeros((P, 4 * P), ml_dtypes.bfloat16)
    maskuT[:, 0:P] = masku_core
    onesT = np.ones((P, 1), ml_dtypes.bfloat16)
    return cosT, sinT, identT, maskuT, onesT


def _in_maps(inputs):
    bf = ml_dtypes.bfloat16
    q = np.asarray(inputs["query"], dtype=np.float32)
    Wq = np.asarray(inputs["Wq"], dtype=np.float32)
    Wk = np.asarray(inputs["Wk"], dtype=np.float32)
    Wv = np.asarray(inputs["Wv"], dtype=np.float32)
    Wo = np.asarray(inputs["Wo"], dtype=np.float32)
    xT = np.ascontiguousarray(q.reshape(ROWS, D).T.astype(bf))
    cosT, sinT, identT, maskuT, onesT = _tables()
    maps = []
    for ci in range(NCORES):
        rs = slice(ci * LF, (ci + 1) * LF)
        maps.append({
            "xT": xT,
            "wqT": np.ascontiguousarray(Wq[rs, :].T.astype(bf)),
            "wkT": np.ascontiguousarray(Wk[rs, :].T.astype(bf)),
            "wvT": np.ascontiguousarray(Wv[rs, :].T.astype(bf)),
            "woT": np.ascontiguousarray(Wo[:, rs].T.astype(bf)),
            "cosT": cosT, "sinT": sinT, "identT": identT,
            "maskuT": maskuT, "onesT": onesT,
        })
    return maps


def _run(inputs, trace=False, **kw):
    global _PROG
    if _PROG is None:
        _PROG = _build()
    res = run_bass_kernel_spmd(_PROG, _in_maps(inputs),
                               core_ids=list(range(NCORES)),
                               trace=trace, **kw)
    acc = np.zeros((ROWS, D), np.float64)
    for r in res.results:
        acc += r["out"].astype(np.float64)
    return acc.astype(np.float32).reshape(B, S, D), res


def kernel(query, Wq, Wk, Wv, Wo):
    out, _ = _run(dict(query=query, Wq=Wq, Wk=Wk, Wv=Wv, Wo=Wo))
    return out


# revision 15
# speedup vs baseline: 1.2263x; 1.0183x over previous
"""Causal attention with RoPE on 8 Trainium2 NeuronCores.

Tensor-parallel over heads: core c owns heads [2c, 2c+2). Each core computes
its heads' Q/K/V projections, RoPE, causal attention in a transposed layout
(keys on partitions), and a partial output projection through its slice of
Wo. The 8 partial outputs are summed on the host.

v2 schedule: the emission is software-pipelined so the in-order PE never
waits on cross-engine latency:
  - Projection matmuls of block i+1 are woven as filler between the
    dependent attention matmuls (scores -> exp -> attn@V) of block i.
  - The causal mask is applied on the PE itself: a small matmul writes
    -1e9 into the upper-triangle strip of the scores PSUM accumulation
    group before the QK matmul, so exp produces exact zeros and no
    vector-engine op sits between exp and attn@V.
  - Softmax denominators come from bf16 exp-sum accumulation on the DVE
    (E += ex per key block) followed by one ones^T @ E matmul per query
    block, replacing the per-block M=1 denominator matmuls on the PE.
  - Q/K projections run as per-(tensor, head) accumulation chains so RoPE
    can start draining each chain while the next accumulates.
PSUM: 2 banks QK ping/pong, 2 banks scores (+denominator), 4 banks shared
ring for attention-out / V projection / Wo tiles.
"""
import numpy as np
import ml_dtypes

import concourse.bacc as bacc
import concourse.bass as bass
import concourse.tile as tile
import concourse.mybir as mybir
from concourse.bass_utils import run_bass_kernel_spmd

AF = mybir.ActivationFunctionType
F32 = mybir.dt.float32
BF16 = mybir.dt.bfloat16

P = 128            # partitions
DH = 128           # head dim
D = 2048           # d_model
S = 2048           # sequence length
B = 2              # batch
NCORES = 8
HL = 2             # heads per core
LF = HL * DH       # 256 local head features
KC = D // P        # 16 d_model chunks
NCB = S // 512     # 4 column blocks of 512 positions per batch
NNT = D // 512     # 4 output column tiles
ROWS = B * S
SCALE = float(1.0 / np.sqrt(DH))

_PROG = None
VARIANT = "base"


def _emit_consts(nc, sbp, t):
    wq = sbp.tile([P, KC * LF], BF16, name="wq")
    wk = sbp.tile([P, KC * LF], BF16, name="wk")
    wv = sbp.tile([P, KC * LF], BF16, name="wv")
    wo = sbp.tile([P, HL * D], BF16, name="wo")
    cos = sbp.tile([DH, S], F32, name="cos")
    sin = sbp.tile([DH, S], F32, name="sin")
    ident = sbp.tile([P, P], BF16, name="ident")
    masku = sbp.tile([P, 4 * P], BF16, name="masku")
    ones = sbp.tile([P, 1], BF16, name="ones")
    for g in range(4):
        gk = slice(g * 4 * P, (g + 1) * 4 * P)
        nc.sync.dma_start(
            out=wq[:, g * 4 * LF:(g + 1) * 4 * LF],
            in_=t["wqT"][gk, :].rearrange("(kc p) f -> p kc f", p=P))
        nc.scalar.dma_start(
            out=wk[:, g * 4 * LF:(g + 1) * 4 * LF],
            in_=t["wkT"][gk, :].rearrange("(kc p) f -> p kc f", p=P))
        nc.gpsimd.dma_start(
            out=wv[:, g * 4 * LF:(g + 1) * 4 * LF],
            in_=t["wvT"][gk, :].rearrange("(kc p) f -> p kc f", p=P))
    nc.scalar.dma_start(out=cos, in_=t["cosT"][:, :])
    nc.scalar.dma_start(out=sin, in_=t["sinT"][:, :])
    nc.gpsimd.dma_start(out=ident, in_=t["identT"][:, :])
    nc.gpsimd.dma_start(out=masku, in_=t["maskuT"][:, :])
    nc.gpsimd.dma_start(out=ones, in_=t["onesT"][:, :])
    for h in range(HL):
        nc.gpsimd.dma_start(
            out=wo[:, h * D:(h + 1) * D],
            in_=t["woT"][h * P:(h + 1) * P, :])
    return dict(wq=wq, wk=wk, wv=wv, wo=wo, cos=cos, sin=sin, ident=ident,
                masku=masku, ones=ones)


class _Filler:
    """Round-robin distributor of independent PE work units (closures)."""

    def __init__(self):
        self.units = []
        self.total_w = 1
        self.acc = 0.0
        self.emitted = 0

    def load(self, units, total_w):
        self.units = list(units)
        self.total_w = max(1, total_w)
        self.acc = 0.0
        self.emitted = 0
        self.n0 = len(self.units)

    def take(self, w):
        self.acc += self.n0 * (w / self.total_w)
        while self.units and self.emitted < self.acc:
            self.units.pop(0)()
            self.emitted += 1

    def drain(self):
        while self.units:
            self.units.pop(0)()


def _emit(nc, sbp, psp, c, t):
    xT, out = t["xT"], t["out"]
    wq, wk, wv, wo = c["wq"], c["wk"], c["wv"], c["wo"]
    cos, sin = c["cos"], c["sin"]
    ident, masku, ones = c["ident"], c["masku"], c["ones"]

    iters = [(b, cb) for b in range(B) for cb in range(NCB)]
    state = {"xt": {}, "qt": {}, "kt": {}, "vsb": {}}

    def emit_xload(i):
        if VARIANT == "noxload" and i >= 2:
            state["xt"][i] = state["xt"][i % 2]
            return
        b, cb = iters[i]
        xtg = []
        for g in range(4):
            xt = sbp.tile([P, 4 * 512], BF16, name=f"xt{b}_{cb}_{g}",
                          tag="xt", bufs=8)
            eng = nc.sync if g % 2 == 0 else nc.gpsimd
            src = xT[g * 4 * P:(g + 1) * 4 * P,
                     b * S + cb * 512: b * S + (cb + 1) * 512]
            eng.dma_start(out=xt, in_=src.rearrange("(kc p) s -> p kc s", p=P))
            xtg.append(xt)
        state["xt"][i] = [xtg[kc // 4][:, (kc % 4) * 512:(kc % 4 + 1) * 512]
                         for kc in range(KC)]

    def build_proj_units(i):
        """48 closures: 32 QK units (2 matmuls each, RoPE on chain end),
        16 V units (2 matmuls each, drain on sub-chain end)."""
        b, cb = iters[i]
        xts = state["xt"][i]
        qt = sbp.tile([P, HL * 512], BF16, name=f"qt{b}_{cb}", tag="qt",
                      bufs=2)
        state["qt"][i] = qt
        if cb == 0:
            state["kt"][b] = sbp.tile([P, HL * S], BF16, name=f"kt{b}",
                                      tag="kt", bufs=2)
            state["vsb"][b] = sbp.tile([P, 4 * NCB * LF], BF16,
                                       name=f"v{b}", tag="vsb", bufs=2)
        kt, vsb = state["kt"][b], state["vsb"][b]
        units = []
        cs = slice(cb * 512, (cb + 1) * 512)

        def mk_rope(pq, dst):
            def rope():
                if VARIANT == "norope":
                    nc.vector.tensor_copy(dst, pq)
                    return
                ra = sbp.tile([P, 512], F32, name="ra", tag="ra", bufs=2)
                nc.vector.tensor_mul(ra, pq, cos[:, cs])
                nc.vector.tensor_mul(dst[0:64, :], pq[64:128, :],
                                     sin[0:64, cs])
                nc.vector.tensor_mul(dst[64:128, :], pq[0:64, :],
                                     sin[64:128, cs])
                nc.vector.tensor_add(dst, dst, ra)
            return rope

        if VARIANT == "noproj":
            nc.sync.dma_start(out=qt[:, 0:512], in_=xts[0])
            nc.sync.dma_start(out=qt[:, 512:1024], in_=xts[1])
            for h in range(HL):
                nc.gpsimd.dma_start(
                    out=kt[:, h * S + cb * 512:h * S + (cb + 1) * 512],
                    in_=xts[2 + h])
            nc.sync.dma_start(out=vsb[:, cb * 4 * LF:cb * 4 * LF + 512],
                              in_=xts[4])
            nc.sync.dma_start(out=vsb[:, cb * 4 * LF + 512:(cb + 1) * 4 * LF],
                              in_=xts[5])
            return units
        for key in ("q", "k"):
            wsb = wq if key == "q" else wk
            for h in range(HL):
                pq = psp.tile([P, 512], F32, name=f"p{key}{h}_{b}_{cb}",
                              tag="qk", bufs=2)
                if key == "q":
                    dst = qt[:, h * 512:(h + 1) * 512]
                else:
                    dst = kt[:, h * S + cb * 512: h * S + (cb + 1) * 512]
                rope = mk_rope(pq, dst)
                for j in range(8):
                    def u(j=j, pq=pq, wsb=wsb, h=h, rope=rope):
                        for kc in (2 * j, 2 * j + 1):
                            nc.tensor.matmul(
                                pq,
                                lhsT=wsb[:, kc * LF + h * DH:
                                         kc * LF + (h + 1) * DH],
                                rhs=xts[kc],
                                start=(kc == 0), stop=(kc == KC - 1))
                        if j == 7:
                            rope()
                    units.append(u)

        for pair in range(2):
            pv = psp.tile([P, 512], F32, name=f"pv{b}_{cb}_{pair}",
                          tag="qk", bufs=2)

            def mk_vdrain(pv=pv, pair=pair):
                def vdrain():
                    kb0 = cb * 4 + pair * 2
                    nc.scalar.copy(
                        vsb[:, kb0 * LF:(kb0 + 2) * LF], pv)
                return vdrain
            vdrain = mk_vdrain()
            for r in (0, 1):
                for j in range(8):
                    def u(j=j, pv=pv, r=r, pair=pair, vdrain=vdrain):
                        gr = pair * 2 + r
                        for kc in (2 * j, 2 * j + 1):
                            nc.tensor.matmul(
                                pv[:, r * 256:(r + 1) * 256],
                                lhsT=xts[kc][:, gr * P:(gr + 1) * P],
                                rhs=wv[:, kc * LF:(kc + 1) * LF],
                                start=(kc == 0), stop=(kc == KC - 1))
                        if r == 1 and j == 7:
                            vdrain()
                    units.append(u)
        return units

    def emit_attention(i, fill):
        """Attention + tail + Wo for iteration i, weaving filler units."""
        b, qj = iters[i]
        nkb = 4 * qj + 4
        qt = state["qt"][i]
        kt, vsb = state["kt"][b], state["vsb"][b]

        po = [psp.tile([P, 512], F32, name=f"po{b}_{h}_{qj}", tag="acc",
                       bufs=3) for h in range(HL)]
        E = sbp.tile([P, HL * 512], BF16, name=f"E{b}_{qj}", tag="E", bufs=2)
        exs = {}

        def _off(kb):
            return max(0, kb - 4 * qj) * P

        def ph1(kb):
            # scores (with PE-side causal mask on the diagonal strip) + exp
            off = _off(kb)
            diag = kb >= 4 * qj
            pss = {}
            for h in range(HL):
                pss[h] = psp.tile([P, 512], F32, name=f"pss{b}_{h}_{qj}_{kb}",
                                  tag="ps", bufs=3)
                lk = kt[:, h * S + kb * P: h * S + (kb + 1) * P]
                if diag:
                    # -1e9 upper-triangle strip (zeros beyond), then
                    # accumulate k^T q on top: exp gives exact 0s, no
                    # vector-engine op between exp and attn@V
                    nc.tensor.matmul(
                        pss[h][:, off:512], lhsT=ident,
                        rhs=masku[:, 0:512 - off],
                        start=True, stop=False)
                    nc.tensor.matmul(
                        pss[h][:, off:512], lhsT=lk,
                        rhs=qt[:, h * 512 + off:(h + 1) * 512],
                        start=False, stop=True)
                else:
                    nc.tensor.matmul(
                        pss[h], lhsT=lk, rhs=qt[:, h * 512:(h + 1) * 512],
                        start=True, stop=True)
            ex = sbp.tile([P, HL * 512], BF16, name=f"ex{b}_{qj}_{kb}",
                          tag="ex", bufs=4)
            fn = AF.Copy if VARIANT == "expcopy" else AF.Exp
            for h in range(HL):
                nc.scalar.activation(ex[:, h * 512 + off:(h + 1) * 512],
                                     pss[h][:, off:512], fn, scale=SCALE)
            exs[kb] = ex

        def ph2(kb):
            # attn@V + denominator partial accumulation
            off = _off(kb)
            ex = exs[kb]
            last = kb == nkb - 1
            for h in range(HL):
                nc.tensor.matmul(
                    po[h][:, off:512],
                    lhsT=vsb[:, kb * LF + h * DH: kb * LF + (h + 1) * DH],
                    rhs=ex[:, h * 512 + off:(h + 1) * 512],
                    start=(kb == 0), stop=last)
            if VARIANT == "noeadd":
                pass
            elif kb == 0:
                nc.vector.tensor_copy(E, ex)
            else:
                nc.vector.tensor_add(E[:, off:512], E[:, off:512],
                                     ex[:, off:512])
                nc.vector.tensor_add(E[:, 512 + off:1024],
                                     E[:, 512 + off:1024],
                                     ex[:, 512 + off:1024])

        lag = 3 if VARIANT == "lag3" else 2
        if VARIANT == "noattn":
            nkb = 0
        for kb in range(nkb):
            ph1(kb)
            fill.take(1)
            if kb >= lag:
                ph2(kb - lag)
        for r in range(lag, 0, -1):
            if nkb - r >= 0:
                fill.take(1)
                ph2(nkb - r)
        fill.take(2)

        # tail: denominators -> reciprocal -> broadcast -> normalize
        skip_tail = VARIANT == "noattn"
        pdd = [psp.tile([1, 512], F32, name=f"pd{b}_{h}_{qj}", tag="ps",
                        bufs=3) for h in range(HL)]
        for h in range(HL):
            if skip_tail:
                break
            nc.tensor.matmul(pdd[h], lhsT=ones,
                             rhs=E[:, h * 512:(h + 1) * 512],
                             start=True, stop=True)
        ot = sbp.tile([P, HL * 512], BF16, name=f"ot{b}_{qj}", tag="ot",
                      bufs=2)
        for h in range(HL):
            if skip_tail:
                break
            recip = sbp.tile([1, 512], F32, name="rc", tag="rc", bufs=2)
            if VARIANT == "noeadd":
                nc.vector.tensor_copy(ot[:, h * 512:(h + 1) * 512], po[h])
                continue
            nc.vector.reciprocal(recip, pdd[h])
            bc = sbp.tile([P, 512], F32, name="bc", tag="bc", bufs=2)
            nc.gpsimd.partition_broadcast(bc, recip)
            nc.vector.tensor_mul(ot[:, h * 512:(h + 1) * 512], po[h], bc)
        # all remaining filler covers the normalize latency and guarantees
        # the woven V-projection drains land before the Wo tiles reuse
        # their PSUM ring slots
        fill.drain()

        # Wo: per 128-row chunk qc, accumulate both heads into pw per nt.
        # Drains rotate across ACT/DVE/Pool so no single engine paces the
        # pw ring; DMA per 1024-col pair from bf16 staging
        for qc in range(4 * qj, 4 * qj + 4):
            st = None
            for nt in range(NNT):
                pw = psp.tile([P, 512], F32, name=f"pw{b}_{qc}_{nt}",
                              tag="acc", bufs=3)
                for h in range(HL):
                    src = vsb if VARIANT == "noattn" else ot
                    nc.tensor.matmul(
                        pw,
                        lhsT=src[:, h * 512 + (qc - 4 * qj) * P:
                                 h * 512 + (qc - 4 * qj + 1) * P],
                        rhs=wo[:, h * D + nt * 512: h * D + (nt + 1) * 512],
                        start=(h == 0), stop=(h == HL - 1))
                if nt % 2 == 0:
                    st = sbp.tile([P, 1024], BF16, name=f"st{b}_{qc}_{nt}",
                                  tag="st", bufs=8)
                dsl = st[:, (nt % 2) * 512:(nt % 2 + 1) * 512]
                if nt % 2 == 0:
                    nc.scalar.copy(dsl, pw)
                else:
                    nc.vector.tensor_copy(dsl, pw)
                if nt % 2 == 1 and VARIANT != "noout":
                    nc.sync.dma_start(
                        out=out[b * S + qc * P: b * S + (qc + 1) * P,
                                (nt - 1) * 512:(nt + 1) * 512],
                        in_=st)

    # ---- prologue: x loads for iterations 0,1; projections for 0 ----
    emit_xload(0)
    emit_xload(1)
    fill = _Filler()
    for u in build_proj_units(0):
        u()
    n = len(iters)
    for i in range(n):
        if i + 2 < n:
            emit_xload(i + 2)
        if i + 1 < n:
            units = build_proj_units(i + 1)
        else:
            units = []
        b, qj = iters[i]
        nkb = 4 * qj + 4
        total_w = nkb + 4
        fill.load(units, total_w)
        emit_attention(i, fill)
        fill.drain()


def _build(loop_n=0, nbody=1):
    nc = bacc.Bacc("TRN2", target_bir_lowering=False, debug=False)
    t = {}
    t["xT"] = nc.dram_tensor("xT", [D, ROWS], BF16, kind="ExternalInput")
    t["wqT"] = nc.dram_tensor("wqT", [D, LF], BF16, kind="ExternalInput")
    t["wkT"] = nc.dram_tensor("wkT", [D, LF], BF16, kind="ExternalInput")
    t["wvT"] = nc.dram_tensor("wvT", [D, LF], BF16, kind="ExternalInput")
    t["woT"] = nc.dram_tensor("woT", [LF, D], BF16, kind="ExternalInput")
    t["cosT"] = nc.dram_tensor("cosT", [DH, S], F32, kind="ExternalInput")
    t["sinT"] = nc.dram_tensor("sinT", [DH, S], F32, kind="ExternalInput")
    t["identT"] = nc.dram_tensor("identT", [P, P], BF16, kind="ExternalInput")
    t["maskuT"] = nc.dram_tensor("maskuT", [P, 4 * P], BF16, kind="ExternalInput")
    t["onesT"] = nc.dram_tensor("onesT", [P, 1], BF16, kind="ExternalInput")
    t["out"] = nc.dram_tensor("out", [ROWS, D], BF16, kind="ExternalOutput")
    with tile.TileContext(nc) as tc:
        with tc.tile_pool(name="sb", bufs=1) as sbp, \
             tc.tile_pool(name="ps", bufs=2, space="PSUM") as psp:
            c = _emit_consts(nc, sbp, t)
            if loop_n:
                with tc.For_i(0, loop_n, 1,
                              hint_engines=(mybir.EngineType.PE,
                                            mybir.EngineType.Activation,
                                            mybir.EngineType.DVE)):
                    for _ in range(nbody):
                        _emit(nc, sbp, psp, c, t)
            else:
                _emit(nc, sbp, psp, c, t)
    nc.compile()
    return nc


def _tables():
    half = np.arange(0, DH, 2, dtype=np.float32) / np.float32(DH)
    inv_freq = (np.float32(1.0) / (np.float32(10000.0) ** half)).astype(np.float32)
    pos = np.arange(S, dtype=np.float32)
    freqs = np.outer(pos, inv_freq).astype(np.float32)        # [S, 64]
    emb = np.concatenate([freqs, freqs], axis=1)              # [S, DH]
    cosT = np.ascontiguousarray(np.cos(emb).T).astype(np.float32)
    sinT = np.sin(emb).T.astype(np.float32).copy()
    sinT[0:DH // 2, :] *= np.float32(-1.0)                    # pre-signed
    sinT = np.ascontiguousarray(sinT)
    identT = np.eye(P, dtype=ml_dtypes.bfloat16)
    # masku[k, c] = -1e9 iff c < k (query c attends only to keys <= c)
    kl = np.arange(P)[:, None]
    cc = np.arange(P)[None, :]
    masku_core = np.where(cc < kl, np.float32(-1e9),
                          np.float32(0.0)).astype(ml_dtypes.bfloat16)
    maskuT = np.zeros((P, 4 * P), ml_dtypes.bfloat16)
    maskuT[:, 0:P] = masku_core
    onesT = np.ones((P, 1), ml_dtypes.bfloat16)
    return cosT, sinT, identT, maskuT, onesT


def _in_maps(inputs):
    bf = ml_dtypes.bfloat16
    q = np.asarray(inputs["query"], dtype=np.float32)
    Wq = np.asarray(inputs["Wq"], dtype=np.float32)
    Wk = np.asarray(inputs["Wk"], dtype=np.float32)
    Wv = np.asarray(inputs["Wv"], dtype=np.float32)
    Wo = np.asarray(inputs["Wo"], dtype=np.float32)
    xT = np.ascontiguousarray(q.reshape(ROWS, D).T.astype(bf))
    cosT, sinT, identT, maskuT, onesT = _tables()
    maps = []
    for ci in range(NCORES):
        rs = slice(ci * LF, (ci + 1) * LF)
        maps.append({
            "xT": xT,
            "wqT": np.ascontiguousarray(Wq[rs, :].T.astype(bf)),
            "wkT": np.ascontiguousarray(Wk[rs, :].T.astype(bf)),
            "wvT": np.ascontiguousarray(Wv[rs, :].T.astype(bf)),
            "woT": np.ascontiguousarray(Wo[:, rs].T.astype(bf)),
            "cosT": cosT, "sinT": sinT, "identT": identT,
            "maskuT": maskuT, "onesT": onesT,
        })
    return maps


def _run(inputs, trace=False, **kw):
    global _PROG
    if _PROG is None:
        _PROG = _build()
    res = run_bass_kernel_spmd(_PROG, _in_maps(inputs),
                               core_ids=list(range(NCORES)),
                               trace=trace, **kw)
    acc = np.zeros((ROWS, D), np.float64)
    for r in res.results:
        acc += r["out"].astype(np.float64)
    return acc.astype(np.float32).reshape(B, S, D), res


def kernel(query, Wq, Wk, Wv, Wo):
    out, _ = _run(dict(query=query, Wq=Wq, Wk=Wk, Wv=Wv, Wo=Wo))
    return out
